# revision 10
# baseline (speedup 1.0000x reference)
"""Bass/Trainium2 kernel for nn_DeformMaxPool2d.

Reference op: x [16,64,256,256] f32, gather_idx [128,128,4] int64 (an exact
permutation of 0..65535 pixel indices). out[b,c,i,j] = max_k x_flat[b,c,idx[i,j,k]].

Strategy: 2D sharding over (pair-groups x out-groups) = G_P x G_O = 8 cores.
Each core owns W = 1024/G_P (b,c) pairs and NO = 16384/G_O output positions.
Host transposes each pair-group's slice to pixel-major xt [65536, W] so one
pixel's W pair-values are one contiguous row (W*4 bytes) — wider rows = fewer
dma_gather descriptors, the dominant HW cost. Device: SWDGE dma_gather pulls
rows xt[idx,:] into SBUF (round-robin over SWDGE queues for overlap), DVE
max-reduces the 4 sources per output, rows DMA back to HBM; host re-permutes.

dma_gather indices are int16, so rows are addressed within two 32768-row
halves of xt. Outputs are class-sorted by low-half source count l (0..4).
To keep the program SPMD-identical across out-groups while letting each
out-group process different outputs, every class is padded to a multiple of
G_O*CH outputs so each out-group gets the same per-class chunk count; the
chunk class schedule (shared by all cores) is baked into the program.
Per chunk of CH same-class outputs:
  gather A: the l low-half sources/output   -> gA [128, n*l, W]
  gather B: the 4-l high-half sources/output-> gB [128, n*(4-l), W]
  reduce A, reduce B (DVE max over sources), tensor_max(A,B) -> o
  DMA o to out rows; host inverts the class-sort at the end.
Gather list order j = (i*K' + k)*128 + p lands row j at partition j%128,
slot j//128 (snake), so output (p,i)'s sources sit at slots i*K'+k — a
fixed-stride view the DVE reduce consumes directly.
"""
import sys
sys.path.insert(0, '/opt/trn_rl_repo')

import numpy as np

B, C, D = 16, 64, 256
HO = 128
K = 4
P = 128
NCORES = 8
NPIX = D * D            # 65536
NOUT = HO * HO          # 16384
PAIRS = B * C           # 1024
HALF = NPIX // 2        # 32768: int16 index range for dma_gather

G_P = 8                 # pair groups (row width W = 128 pairs = 512B)
G_O = 1                 # out groups  (all outputs on every core)
W = PAIRS // G_P
NO = NOUT // G_O

N_PER_CHUNK = 2         # outputs per partition per chunk
CH = P * N_PER_CHUNK    # 256 outputs per chunk -> gathers <= 1024 idx
NQ = 1                  # single SWDGE queue (multi-queue measured 4x slower)
BUFS = 6                # in-flight chunk sets (SBUF-bounded)
GRP = 4                 # chunks whose same-half gathers are issued back-to-back


def _wrap16(lst):
    """dma_gather index layout: element j -> partition j%16, col j//16,
    replicated across the 8 gpsimd groups -> [128, len/16] int16."""
    a = np.asarray(lst, np.int16).reshape(-1, 16).T       # [16, len/16]
    return np.tile(a, (8, 1))


def make_plan(gather_idx):
    """Host planning: class-sort outputs, pad classes to G_O*CH multiples,
    deal chunks across out-groups, build per-out-group gather lists.

    Returns (idx_devs [G_O] of [128, totcols] int16, sched [nchunks] of l,
             out_perms [G_O] of [tot_rows] (output id or -1), tot_rows).
    """
    n = N_PER_CHUNK
    g4 = np.asarray(gather_idx).reshape(NOUT, K).astype(np.int64)
    lcnt = (g4 < HALF).sum(axis=1)                        # [NOUT] 0..4
    dummy_rows = {l: np.array([0] * l + [HALF] * (K - l), np.int64)
                  for l in range(K + 1)}
    # per out-group accumulators
    cols_blocks = [[] for _ in range(G_O)]
    out_perm = [[] for _ in range(G_O)]
    sched = []          # per chunk: (l, aoff, acols, boff, bcols)
    col = 0
    for l in range(K + 1):
        ids = np.nonzero(lcnt == l)[0]
        npad = (-len(ids)) % (G_O * CH)
        if len(ids) + npad == 0:
            continue
        ids_p = np.concatenate([ids, np.full(npad, -1, np.int64)])
        nch_per_grp = len(ids_p) // (G_O * CH)
        for ci in range(nch_per_grp):
            acols = (n * l * P) // 16
            bcols = (n * (K - l) * P) // 16
            sched.append((l, col, acols, col + acols, bcols))
            col += acols + bcols
        # deal: out-group g gets chunks [g*nch : (g+1)*nch]
        for g in range(G_O):
            seg = ids_p[g * nch_per_grp * CH:(g + 1) * nch_per_grp * CH]
            for c0 in range(0, len(seg), CH):
                blk = seg[c0:c0 + CH]
                rows = np.empty((CH, K), np.int64)
                real = blk >= 0
                rows[real] = g4[blk[real]]
                rows[~real] = dummy_rows[l]
                order = np.argsort(rows >= HALF, axis=1, kind="stable")
                rows = np.take_along_axis(rows, order, axis=1)
                low = rows[:, :l]
                high = rows[:, l:] - HALF

                def mklist(src):                          # src [CH, K']
                    kk = src.shape[1]
                    s = src.reshape(P, n, kk)             # [p, i, k]
                    return s.transpose(1, 2, 0).reshape(-1)
                if l > 0:
                    cols_blocks[g].append(_wrap16(mklist(low)))
                if l < K:
                    cols_blocks[g].append(_wrap16(mklist(high)))
                out_perm[g].append(blk)
    idx_devs = [np.ascontiguousarray(np.concatenate(cb, axis=1))
                for cb in cols_blocks]
    out_perms = [np.concatenate(op) for op in out_perm]
    tot_rows = len(out_perms[0])
    assert all(len(op) == tot_rows for op in out_perms)
    return idx_devs, sched, out_perms, tot_rows


def build_program(sched, totcols, tot_rows, repeats=1, bufs=BUFS):
    import concourse.bacc as bacc
    import concourse.tile as tile
    from concourse import mybir

    n = N_PER_CHUNK
    nc = bacc.Bacc("TRN2", num_swdge_queues=NQ)
    xlo_d = nc.dram_tensor("xlo", [HALF, W], mybir.dt.float32, kind="ExternalInput")
    xhi_d = nc.dram_tensor("xhi", [NPIX - HALF, W], mybir.dt.float32,
                           kind="ExternalInput")
    idx_d = nc.dram_tensor("idx", [P, totcols], mybir.dt.int16, kind="ExternalInput")
    out_d = nc.dram_tensor("out", [tot_rows, W], mybir.dt.float32,
                           kind="ExternalOutput")

    with tile.TileContext(nc) as tc:
        with tc.tile_pool(name="sbuf", bufs=1) as ipool, \
             tc.tile_pool(name="g", bufs=2 * GRP) as gpool, \
             tc.tile_pool(name="r", bufs=4) as rpool, \
             tc.tile_pool(name="o", bufs=2 * GRP) as opool:
            idx_t = ipool.tile([P, totcols], mybir.dt.int16)
            nc.sync.dma_start(out=idx_t[:], in_=idx_d[:])
            out_view = out_d[:].rearrange("(c p n) d -> c p n d", p=P, n=n)
            for _ in range(repeats):
                for g0 in range(0, len(sched), GRP):
                    grp = sched[g0:g0 + GRP]
                    gAs, gBs = {}, {}
                    # all low-half gathers of the group back-to-back ...
                    for j, (l, aoff, acols, boff, bcols) in enumerate(grp):
                        if l > 0:
                            nA = n * l * P
                            gA = gpool.tile([P, n * l, W], mybir.dt.float32,
                                            tag="gA")
                            nc.gpsimd.dma_gather(
                                gA[:], xlo_d[:], idx_t[:, aoff:aoff + acols],
                                nA, nA, W,
                            )
                            gAs[j] = gA
                    # ... then all high-half gathers (fewer source switches)
                    for j, (l, aoff, acols, boff, bcols) in enumerate(grp):
                        if l < K:
                            nB = n * (K - l) * P
                            gB = gpool.tile([P, n * (K - l), W],
                                            mybir.dt.float32, tag="gB")
                            nc.gpsimd.dma_gather(
                                gB[:], xhi_d[:], idx_t[:, boff:boff + bcols],
                                nB, nB, W,
                            )
                            gBs[j] = gB
                    for j, (l, aoff, acols, boff, bcols) in enumerate(grp):
                        o = opool.tile([P, n, W], mybir.dt.float32, tag="o")
                        rA = rB = None
                        if l > 0:
                            dstA = o if l == K else rpool.tile(
                                [P, n, W], mybir.dt.float32, tag="rA")
                            nc.vector.tensor_reduce(
                                out=dstA[:],
                                in_=gAs[j][:].rearrange(
                                    "p (n k) d -> p n d k", k=l),
                                axis=mybir.AxisListType.X,
                                op=mybir.AluOpType.max,
                            )
                            rA = dstA
                        if l < K:
                            dstB = o if l == 0 else rpool.tile(
                                [P, n, W], mybir.dt.float32, tag="rB")
                            nc.vector.tensor_reduce(
                                out=dstB[:],
                                in_=gBs[j][:].rearrange(
                                    "p (n k) d -> p n d k", k=K - l),
                                axis=mybir.AxisListType.X,
                                op=mybir.AluOpType.max,
                            )
                            rB = dstB
                        if 0 < l < K:
                            nc.vector.tensor_max(o[:], rA[:], rB[:])
                        nc.sync.dma_start(out=out_view[g0 + j], in_=o[:])
    nc.compile()
    return nc


def shard_inputs(x):
    """Per pair-group transposed halves; core (og, pg) uses pair-group pg."""
    xf = np.asarray(x).reshape(PAIRS, NPIX)
    groups = []
    for pg in range(G_P):
        sl = xf[pg * W:(pg + 1) * W]
        xt = np.ascontiguousarray(sl.T)                   # [NPIX, W]
        groups.append((xt[:HALF], np.ascontiguousarray(xt[HALF:])))
    return groups


def assemble_output(results, out_perms):
    full = np.empty((PAIRS, NOUT), np.float32)
    for core in range(NCORES):
        og, pg = divmod(core, G_P)
        perm = out_perms[og]
        valid = perm >= 0
        dev = np.asarray(results[core]["out"])            # [tot_rows, W]
        full[pg * W:(pg + 1) * W, perm[valid]] = dev[valid].T
    return np.ascontiguousarray(full.reshape(B, C, HO, HO))


_cache = {}


def prepare(gather_idx, repeats=1):
    key = ("plan", gather_idx.shape, int(np.asarray(gather_idx)[0, 0, 0]),
           repeats)
    if key not in _cache:
        idx_devs, sched, out_perms, tot_rows = make_plan(gather_idx)
        nc = build_program(sched, idx_devs[0].shape[1], tot_rows,
                           repeats=repeats)
        _cache[key] = (idx_devs, nc, out_perms)
    return _cache[key]


def kernel(x, gather_idx):
    from concourse.bass_utils import run_bass_kernel_spmd
    idx_devs, nc, out_perms = prepare(gather_idx)
    groups = shard_inputs(x)
    in_maps = []
    for core in range(NCORES):
        og, pg = divmod(core, G_P)
        lo, hi = groups[pg]
        in_maps.append({"xlo": lo, "xhi": hi, "idx": idx_devs[og]})
    res = run_bass_kernel_spmd(nc, in_maps, list(range(NCORES)))
    return assemble_output(res.results, out_perms)
